# revision 1
# baseline (speedup 1.0000x reference)
"""Causal multi-head attention (B=2, S=2048, D=1024, H=16, hd=64) on 8 trn2 cores.

Sharding: core c handles batch b = c//4 and head group hg = c%4 (4 heads each).
Each core computes its Q/K/V shard (tensor-parallel columns of W_qkv), causal
attention for its 4 heads with scores held transposed ([s_k, s_q] so the PV
matmul needs no on-chip transposes), and a partial output projection over its
256 rows of W_proj. The host sums the 4 partials per batch and adds the exact
bias terms (softmax rows sum to 1, so attn@(V + 1 bv^T) = attn@V + bv^T; the
b_qkv V-slice and b_proj are applied on the host).

Matmul operands are bf16 (PSUM accumulation fp32); x is transposed/cast on the
host as part of sharding so the device needs no transposes at all.
"""

import numpy as np
import ml_dtypes
from contextlib import ExitStack

B, S, D, H = 2, 2048, 1024, 16
HD = 64
NCORES = 8
FPC = 256  # features per core (4 heads x 64)

_CACHE = {}


def _build():
    import concourse.bacc as bacc
    import concourse.tile as tile
    import concourse.mybir as mybir

    f32 = mybir.dt.float32
    bf16 = mybir.dt.bfloat16
    AF = mybir.ActivationFunctionType

    nc = bacc.Bacc("TRN2", target_bir_lowering=False, debug=False, num_devices=NCORES)

    xT = nc.dram_tensor("xT", [D, S], bf16, kind="ExternalInput").ap()
    wq = nc.dram_tensor("wq", [D, FPC], bf16, kind="ExternalInput").ap()
    wk = nc.dram_tensor("wk", [D, FPC], bf16, kind="ExternalInput").ap()
    wv = nc.dram_tensor("wv", [D, FPC], bf16, kind="ExternalInput").ap()
    wp = nc.dram_tensor("wp", [FPC, D], bf16, kind="ExternalInput").ap()
    bqk = nc.dram_tensor("bqk", [128, 4], f32, kind="ExternalInput").ap()
    maskT = nc.dram_tensor("maskT", [128, 128], bf16, kind="ExternalInput").ap()
    out = nc.dram_tensor("out", [S, D], f32, kind="ExternalOutput").ap()

    with tile.TileContext(nc) as tc:
        with ExitStack() as ctx:
            _body(ctx, tc, mybir, out, xT, wq, wk, wv, wp, bqk, maskT)

    nc.compile()
    return nc


def _body(ctx, tc, mybir, out, xT, wq, wk, wv, wp, bqk, maskT):
    nc = tc.nc
    f32 = mybir.dt.float32
    bf16 = mybir.dt.bfloat16
    AF = mybir.ActivationFunctionType
    NK = D // 128   # 8 contraction tiles for qkv/proj-input dim
    NS = S // 128   # 16 sequence tiles

    sb = ctx.enter_context(tc.tile_pool(name="sb", bufs=1))

    xt_a = sb.tile([128, NK * S], bf16, name="xta", tag="xta")
    xt_t = [xt_a[:, k * S:(k + 1) * S] for k in range(NK)]
    wq_a = sb.tile([128, NK * FPC], bf16, name="wqa", tag="wqa")
    wq_t = [wq_a[:, k * FPC:(k + 1) * FPC] for k in range(NK)]
    wk_a = sb.tile([128, NK * FPC], bf16, name="wka", tag="wka")
    wk_t = [wk_a[:, k * FPC:(k + 1) * FPC] for k in range(NK)]
    wv_a = sb.tile([128, NK * FPC], bf16, name="wva", tag="wva")
    wv_t = [wv_a[:, k * FPC:(k + 1) * FPC] for k in range(NK)]
    wp_a = sb.tile([128, 2 * D], bf16, name="wpa", tag="wpa")
    wp_t = [wp_a[:, k * D:(k + 1) * D] for k in range(2)]
    qt_t = [sb.tile([128, S], bf16, name=f"qtt{f}", tag=f"qtt{f}") for f in range(2)]
    kt_t = [sb.tile([128, S], bf16, name=f"ktt{f}", tag=f"ktt{f}") for f in range(2)]
    v_t = [sb.tile([128, 4 * 65], bf16, name=f"vt{s}", tag=f"vt{s}") for s in range(NS)]
    ot_t = [sb.tile([128, S], bf16, name=f"ott{f}", tag=f"ott{f}") for f in range(2)]
    bqk_t = sb.tile([128, 4], f32, name="bqkt", tag="bqkt")
    mask_t = sb.tile([128, 128], bf16, name="maskt", tag="maskt")

    p_pool = ctx.enter_context(tc.tile_pool(name="pp", bufs=4))
    rc_pool = ctx.enter_context(tc.tile_pool(name="rcp", bufs=2))
    oo_pool = ctx.enter_context(tc.tile_pool(name="oop", bufs=3))

    # ---- input DMAs, few big transfers, critical-path first: the first
    # attention pair needs only bqk/mask, wq/wk and the s<1024 half of x^T
    nc.sync.dma_start(bqk_t[:], bqk[:])
    nc.sync.dma_start(mask_t[:], maskT[:])
    nc.sync.dma_start(wq_a.rearrange("p (k f) -> p k f", k=NK),
                      wq.rearrange("(k p) f -> p k f", p=128))
    nc.sync.dma_start(wk_a.rearrange("p (k f) -> p k f", k=NK),
                      wk.rearrange("(k p) f -> p k f", p=128))
    xt3 = xt_a.rearrange("p (k s) -> p k s", k=NK)
    xs3 = xT.rearrange("(k p) s -> p k s", p=128)
    nc.sync.dma_start(xt3[:, :, 0:1024], xs3[:, :, 0:1024])
    nc.sync.dma_start(wv_a.rearrange("p (k f) -> p k f", k=NK),
                      wv.rearrange("(k p) f -> p k f", p=128))
    nc.sync.dma_start(xt3[:, :, 1024:2048], xs3[:, :, 1024:2048])
    nc.sync.dma_start(wp_a.rearrange("p (k f) -> p k f", k=2),
                      wp.rearrange("(k p) f -> p k f", p=128))

    # Unified PSUM pools for every phase (no phase barriers): "sc" slots are
    # 2 banks each x2, "pv" slots 2 banks each x2 -> 8 banks total.
    scp = ctx.enter_context(tc.tile_pool(name="ps_sc", bufs=2, space="PSUM"))
    pvp = ctx.enter_context(tc.tile_pool(name="ps_pv", bufs=2, space="PSUM"))

    def qkt_group(dst, w_t, bcol, f, c2):
        """One [128,1024] accumulation group of the Q^T/K^T projection."""
        ps = scp.tile([128, 1024], f32, name="sc", tag="sc", bufs=2)
        for k in range(NK):
            for sp in range(2):
                nc.tensor.matmul(
                    ps[:, sp * 512:(sp + 1) * 512],
                    w_t[k][:, f * 128:(f + 1) * 128],
                    xt_t[k][:, c2 * 1024 + sp * 512: c2 * 1024 + (sp + 1) * 512],
                    start=(k == 0), stop=(k == NK - 1),
                )
        nc.vector.tensor_scalar_add(
            dst[f][:, c2 * 1024:(c2 + 1) * 1024], ps[:],
            bqk_t[:, bcol + f: bcol + f + 1],
        )

    def v_group(s):
        psv = scp.tile([128, FPC], f32, name="sc", tag="sc", bufs=2)
        for k in range(NK):
            nc.tensor.matmul(
                psv[:],
                xt_t[k][:, s * 128:(s + 1) * 128],
                wv_t[k][:],
                start=(k == 0), stop=(k == NK - 1),
            )
        v3 = v_t[s].rearrange("p (h c) -> p h c", h=4)
        nc.vector.tensor_copy(v3[:, :, 0:64],
                              psv.rearrange("p (h c) -> p h c", h=4)[:])
        nc.vector.memset(v3[:, :, 64:65], 1.0)

    class AttnUnit:
        """Causal attention for head h over queries [half*1024, +1024)."""

        def __init__(self, h, half):
            self.h, self.half = h, half
            self.hp, self.hh = h // 2, h % 2
            self.r0 = self.hh * 64
            self.q0 = half * 1024
            self.ki_n = NS // 2 * (half + 1)
            self.pv = pvp.tile([128, 1024], f32, name="pv", tag="pv", bufs=2)

        def emit_scores(self, ki):
            q0, r0 = self.q0, self.r0
            qt, kt = qt_t[self.hp], kt_t[self.hp]
            qs = max(ki * 128, q0)   # first unmasked q for this k block
            a0 = qs - q0             # local col offset in the 1024 tile
            self.a0 = a0
            self.diag = ki * 128 >= q0   # diagonal block lives in this half
            self.spans = [(a0, 512), (512, 1024)] if a0 < 512 else [(a0, 1024)]
            self.sc = scp.tile([128, 1024], f32, name="sc", tag="sc", bufs=2)
            for (a, b) in self.spans:
                nc.tensor.matmul(
                    self.sc[:, a:b],
                    kt[r0:r0 + 64, ki * 128:(ki + 1) * 128],
                    qt[r0:r0 + 64, q0 + a:q0 + b],
                    start=True, stop=True,
                )

        def emit_exp(self, ki):
            a0 = self.a0
            self.P = p_pool.tile([128, 1024], bf16, name="P", tag="P", bufs=6)
            nc.scalar.activation(self.P[:, a0:1024], self.sc[:, a0:1024], AF.Exp,
                                 scale=float(HD) ** -0.5)
            if self.diag:  # causal mask on the diagonal block
                nc.vector.tensor_mul(self.P[:, a0:a0 + 128],
                                     self.P[:, a0:a0 + 128], mask_t[:])

        def emit_pv(self, ki):
            for (a, b) in self.spans:
                # last k-block contributing to this psum bank
                last_ki = min(self.ki_n - 1, (self.q0 + b - 1) // 128)
                nc.tensor.matmul(
                    self.pv[0:65, a:b],
                    v_t[ki][:, self.h * 65:self.h * 65 + 65],
                    self.P[:, a:b],
                    start=(ki == 0), stop=(ki == last_ki),
                )

        def finish(self):
            pv = self.pv
            dcp = rc_pool.tile([1, 1024], f32, name="dcp", tag="dcp", bufs=2)
            nc.vector.tensor_copy(dcp[:], pv[64:65, 0:1024])
            rcp = rc_pool.tile([1, 1024], f32, name="rcp", tag="rcp", bufs=2)
            nc.vector.reciprocal_approx_fast(rcp[:], dcp[:])
            rbc = rc_pool.tile([64, 1024], f32, name="rbc", tag="rbc", bufs=2)
            nc.gpsimd.partition_broadcast(rbc[:], rcp[:], channels=64)
            nc.vector.tensor_mul(
                ot_t[self.hp][self.r0:self.r0 + 64, self.q0:self.q0 + 1024],
                pv[0:64, :], rbc[:],
            )

    def attn_pair(ha, hb, half, fillers=()):
        """Two heads' units interleaved at ki granularity (two chains in
        flight hide the scores->exp->PV latency). fillers[ki] is a list of
        thunks emitting independent PE work woven between iterations."""
        ua, ub = AttnUnit(ha, half), AttnUnit(hb, half)
        for ki in range(ua.ki_n):
            # adjacent scores MMs land in different PE row groups (heads at
            # partition 0 and 64) and execute concurrently in the array
            ua.emit_scores(ki)
            ub.emit_scores(ki)
            ua.emit_exp(ki)
            ub.emit_exp(ki)
            ua.emit_pv(ki)
            ub.emit_pv(ki)
            if ki < len(fillers):
                for fn in fillers[ki]:
                    fn()
        ua.finish()
        ub.finish()

    oo_box = {}

    def proj_group(s):
        pj = scp.tile([128, 1024], f32, name="sc", tag="sc", bufs=2)
        for nh in range(2):
            for k2 in range(2):
                nc.tensor.matmul(
                    pj[:, nh * 512:(nh + 1) * 512],
                    ot_t[k2][:, s * 128:(s + 1) * 128],
                    wp_t[k2][:, nh * 512:(nh + 1) * 512],
                    start=(k2 == 0), stop=(k2 == 1),
                )
        if s % 2 == 0:
            oo_box[0] = oo_pool.tile([128, 2 * D], f32, name="oo", tag="oo",
                                     bufs=2)
        oo = oo_box[0]
        nc.vector.tensor_copy(oo[:, (s % 2) * D:(s % 2 + 1) * D], pj[:])
        if s % 2 == 1:
            nc.sync.dma_start(
                out[(s - 1) * 128:(s + 1) * 128, :].rearrange(
                    "(g p) n -> p g n", p=128),
                oo.rearrange("p (g n) -> p g n", g=2))

    # Program order = scheduler priority. Prelude computes the f0 tiles of
    # Q^T/K^T plus all of V (PE-dense, warms HAM); the f1 tiles are emitted
    # as PE filler between the first attention stages (which are ACT-paced);
    # proj of a finished q-half fills the last stage's gaps.
    from functools import partial

    qkt_group(qt_t, wq_t, 0, 0, 0)
    qkt_group(kt_t, wk_t, 2, 0, 0)
    qkt_group(qt_t, wq_t, 0, 0, 1)
    qkt_group(kt_t, wk_t, 2, 0, 1)
    v_group(0)
    v_group(1)

    # weave V[2..7] (needed by this pair one ki ahead) and the f1 c2=0
    # Q^T/K^T groups (needed by the NEXT pair) into the first pair
    attn_pair(0, 1, 0, fillers=[
        [partial(v_group, 2), partial(v_group, 3)],
        [partial(v_group, 4), partial(v_group, 5)],
        [partial(v_group, 6), partial(v_group, 7)],
        [partial(qkt_group, qt_t, wq_t, 0, 1, 0)],
        [partial(qkt_group, kt_t, wk_t, 2, 1, 0)],
    ])
    # V[8..15] (needed by the half-1 pairs) woven into the second pair
    attn_pair(2, 3, 0, fillers=[
        [partial(v_group, 8 + ki)] for ki in range(8)])
    # f1 c2=1 (needed by pair(2,3,1)) + proj of the finished half 0
    attn_pair(0, 1, 1, fillers=[
        [partial(qkt_group, qt_t, wq_t, 0, 1, 1)],
        [partial(qkt_group, kt_t, wk_t, 2, 1, 1)],
        [partial(proj_group, 0)],
        [partial(proj_group, 1)],
        [partial(proj_group, 2)],
    ])
    attn_pair(2, 3, 1, fillers=[
        [partial(proj_group, s + 3)] if s < 5 else [] for s in range(16)])
    for s in range(8, NS):
        proj_group(s)


def _in_maps(x, W_qkv, b_qkv, W_proj):
    bf = ml_dtypes.bfloat16
    maps = []
    # multiplicative causal mask for the transposed diag block: keep k<=q
    mask = np.triu(np.ones((128, 128), np.float32)).astype(bf)
    for core in range(NCORES):
        b, hg = core // 4, core % 4
        cs = slice(hg * FPC, (hg + 1) * FPC)
        bq = b_qkv[cs].astype(np.float32)
        bk = b_qkv[D + hg * FPC: D + (hg + 1) * FPC].astype(np.float32)
        maps.append({
            "xT": np.ascontiguousarray(x[b].T).astype(bf),
            "wq": np.ascontiguousarray(W_qkv[:, cs]).astype(bf),
            "wk": np.ascontiguousarray(W_qkv[:, D + hg * FPC: D + (hg + 1) * FPC]).astype(bf),
            "wv": np.ascontiguousarray(W_qkv[:, 2 * D + hg * FPC: 2 * D + (hg + 1) * FPC]).astype(bf),
            "wp": np.ascontiguousarray(W_proj[hg * FPC:(hg + 1) * FPC, :]).astype(bf),
            "bqk": np.ascontiguousarray(
                np.stack([bq[0:128], bq[128:256], bk[0:128], bk[128:256]], axis=1)),
            "maskT": mask,
        })
    return maps


def get_nc():
    if "nc" not in _CACHE:
        _CACHE["nc"] = _build()
    return _CACHE["nc"]


def _postprocess(partials, b_qkv, W_proj, b_proj):
    out = np.zeros((B, S, D), np.float32)
    for core in range(NCORES):
        out[core // 4] += partials[core]
    bv = np.asarray(b_qkv, np.float32)[2 * D:3 * D]
    out += bv @ np.asarray(W_proj, np.float32) + np.asarray(b_proj, np.float32)
    return out


def kernel(x, W_qkv, b_qkv, W_proj, b_proj, _trace=False):
    from concourse.bass_utils import run_bass_kernel_spmd

    x = np.asarray(x, np.float32)
    W_qkv = np.asarray(W_qkv, np.float32)
    b_qkv = np.asarray(b_qkv, np.float32)
    W_proj = np.asarray(W_proj, np.float32)
    b_proj = np.asarray(b_proj, np.float32)

    nc = get_nc()
    maps = _in_maps(x, W_qkv, b_qkv, W_proj)
    res = run_bass_kernel_spmd(nc, maps, list(range(NCORES)), trace=_trace)
    _CACHE["last_result"] = res
    partials = [res.results[c]["out"] for c in range(NCORES)]
    return _postprocess(partials, b_qkv, W_proj, b_proj)



# revision 13
# speedup vs baseline: 1.0156x; 1.0156x over previous
"""Causal multi-head attention (B=2, S=2048, D=1024, H=16, hd=64) on 8 trn2 cores.

Sharding: core c handles batch b = c//4 and head group hg = c%4 (4 heads each).
Each core computes its Q/K/V shard (tensor-parallel columns of W_qkv), causal
attention for its 4 heads with scores held transposed ([s_k, s_q] so the PV
matmul needs no on-chip transposes), and a partial output projection over its
256 rows of W_proj. The host sums the 4 partials per batch and adds the exact
bias terms (softmax rows sum to 1, so attn@(V + 1 bv^T) = attn@V + bv^T; the
b_qkv V-slice and b_proj are applied on the host).

v2 schedule:
- input DMAs are chunked per contraction k-tile and ordered so the first QKV
  matmul can start ~1.5us in; dummy matmuls on the mask tile warm the PE HAM
  clock gate during the DMA ramp.
- attention pairs software-pipeline the PV matmuls one ki behind scores/exp,
  so the next iteration's scores (and hence exp) never queue behind a PV that
  is waiting on the current exp: the ACT engine stays saturated.
- PE filler work (remaining QKV projections, V groups, output projection)
  is emitted between scores and the lagged PV in <=1.7us chunks.
- each pair's PV psum columns 0:512 stop accumulating 4 ki early, so the
  normalize (finish) is split per column half and the output projection of
  finished q-tiles weaves into the *last* attention pair instead of
  serializing after it.
- proj results are cast to bf16 (host accumulates in f32), halving out-DMA.
"""

import numpy as np
import ml_dtypes
from contextlib import ExitStack
from functools import partial

B, S, D, H = 2, 2048, 1024, 16
HD = 64
NCORES = 8
FPC = 256  # features per core (4 heads x 64)

_CACHE = {}


def _build():
    import concourse.bacc as bacc
    import concourse.tile as tile
    import concourse.mybir as mybir

    f32 = mybir.dt.float32
    bf16 = mybir.dt.bfloat16

    nc = bacc.Bacc("TRN2", target_bir_lowering=False, debug=False, num_devices=NCORES)

    xT = nc.dram_tensor("xT", [D, S], bf16, kind="ExternalInput").ap()
    wqkv = nc.dram_tensor("wqkv", [D, 3 * FPC], bf16, kind="ExternalInput").ap()
    wp = nc.dram_tensor("wp", [FPC, D], bf16, kind="ExternalInput").ap()
    bqk = nc.dram_tensor("bqk", [128, 4], f32, kind="ExternalInput").ap()
    maskT = nc.dram_tensor("maskT", [128, 128], bf16, kind="ExternalInput").ap()
    out = nc.dram_tensor("out", [S, D], bf16, kind="ExternalOutput").ap()

    with tile.TileContext(nc) as tc:
        with ExitStack() as ctx:
            _body(ctx, tc, mybir, out, xT, wqkv, wp, bqk, maskT)

    nc.compile()
    return nc


def _body(ctx, tc, mybir, out, xT, wqkv, wp, bqk, maskT):
    nc = tc.nc
    f32 = mybir.dt.float32
    bf16 = mybir.dt.bfloat16
    AF = mybir.ActivationFunctionType
    NK = D // 128   # 8 contraction tiles for qkv/proj-input dim
    NS = S // 128   # 16 sequence tiles

    sb = ctx.enter_context(tc.tile_pool(name="sb", bufs=1))

    xt_a = sb.tile([128, NK * S], bf16, name="xta", tag="xta")
    xt_t = [xt_a[:, k * S:(k + 1) * S] for k in range(NK)]
    w_a = sb.tile([128, NK * 3 * FPC], bf16, name="wa", tag="wa")
    wq_t = [w_a[:, k * 3 * FPC: k * 3 * FPC + FPC] for k in range(NK)]
    wk_t = [w_a[:, k * 3 * FPC + FPC: k * 3 * FPC + 2 * FPC] for k in range(NK)]
    wv_t = [w_a[:, k * 3 * FPC + 2 * FPC: (k + 1) * 3 * FPC] for k in range(NK)]
    wp_a = sb.tile([128, 2 * D], bf16, name="wpa", tag="wpa")
    wp_t = [wp_a[:, k * D:(k + 1) * D] for k in range(2)]
    qt_t = [sb.tile([128, S], bf16, name=f"qtt{f}", tag=f"qtt{f}") for f in range(2)]
    kt_t = [sb.tile([128, S], bf16, name=f"ktt{f}", tag=f"ktt{f}") for f in range(2)]
    v_t = [sb.tile([128, 4 * 65], bf16, name=f"vt{s}", tag=f"vt{s}") for s in range(NS)]
    ot_t = [sb.tile([128, S], bf16, name=f"ott{f}", tag=f"ott{f}") for f in range(2)]
    bqk_t = sb.tile([128, 4], f32, name="bqkt", tag="bqkt")
    mask_t = sb.tile([128, 128], bf16, name="maskt", tag="maskt")

    p_pool = ctx.enter_context(tc.tile_pool(name="pp", bufs=6))
    rc_pool = ctx.enter_context(tc.tile_pool(name="rcp", bufs=2))
    oo_pool = ctx.enter_context(tc.tile_pool(name="oop", bufs=3))

    # ---- input DMAs: chunked per k-tile, in the order compute consumes them.
    nc.sync.dma_start(bqk_t[:], bqk[:])
    nc.sync.dma_start(mask_t[:], maskT[:])
    w3 = wqkv.rearrange("(k p) f -> p k f", p=128)
    x3 = xT.rearrange("(k p) s -> p k s", p=128)
    wa3 = w_a.rearrange("p (k f) -> p k f", k=NK)
    xa3 = xt_a.rearrange("p (k s) -> p k s", k=NK)
    for k in range(NK):
        nc.sync.dma_start(wa3[:, k:k + 1, :], w3[:, k:k + 1, :])
        nc.sync.dma_start(xa3[:, k:k + 1, 0:1024], x3[:, k:k + 1, 0:1024])
    for k in range(NK):
        nc.sync.dma_start(xa3[:, k:k + 1, 1024:2048], x3[:, k:k + 1, 1024:2048])
    nc.sync.dma_start(wp_a.rearrange("p (k f) -> p k f", k=2),
                      wp.rearrange("(k p) f -> p k f", p=128))

    # PSUM: "sc" slots 2 banks x2 (scores/qkv/v/proj rotation), "pv" 2 banks x2.
    scp = ctx.enter_context(tc.tile_pool(name="ps_sc", bufs=2, space="PSUM"))
    pvp = ctx.enter_context(tc.tile_pool(name="ps_pv", bufs=2, space="PSUM"))

    # ---- HAM warmup: dummy matmuls on the mask tile keep the PE busy (and
    # the clock gate open) while the first input chunks stream in. The warm
    # tile borrows a pv-pool slot (unused until the first attention pair,
    # by which point all dummies have long retired).
    def warm(n):
        warm_t = pvp.tile([128, 1024], f32, name="pv", tag="pv", bufs=2)
        for _ in range(n):
            nc.tensor.matmul(warm_t[:, 0:128], mask_t[:], mask_t[:],
                             start=True, stop=True)

    def qkt_sp(dst, w_t, bcol, f, c2, sp, interleave=0):
        """Half (512 q cols) of one [128,1024] Q^T/K^T projection group."""
        ps = scp.tile([128, 1024], f32, name="sc", tag="sc", bufs=2)
        for k in range(NK):
            if interleave:
                warm(interleave)
            nc.tensor.matmul(
                ps[:, sp * 512:(sp + 1) * 512],
                w_t[k][:, f * 128:(f + 1) * 128],
                xt_t[k][:, c2 * 1024 + sp * 512: c2 * 1024 + (sp + 1) * 512],
                start=(k == 0), stop=(k == NK - 1),
            )
        nc.vector.tensor_scalar_add(
            dst[f][:, c2 * 1024 + sp * 512:c2 * 1024 + (sp + 1) * 512],
            ps[:, sp * 512:(sp + 1) * 512],
            bqk_t[:, bcol + f: bcol + f + 1],
        )

    def v_group(s):
        psv = scp.tile([128, FPC], f32, name="sc", tag="sc", bufs=2)
        for k in range(NK):
            nc.tensor.matmul(
                psv[:],
                xt_t[k][:, s * 128:(s + 1) * 128],
                wv_t[k][:],
                start=(k == 0), stop=(k == NK - 1),
            )
        v3 = v_t[s].rearrange("p (h c) -> p h c", h=4)
        nc.vector.tensor_copy(v3[:, :, 0:64],
                              psv.rearrange("p (h c) -> p h c", h=4)[:])
        nc.vector.memset(v3[:, :, 64:65], 1.0)

    class AttnUnit:
        """Causal attention for head h over queries [half*1024, +1024)."""

        def __init__(self, h, half):
            self.h, self.half = h, half
            self.hp, self.hh = h // 2, h % 2
            self.r0 = self.hh * 64
            self.q0 = half * 1024
            self.ki_n = NS // 2 * (half + 1)
            self.pv = pvp.tile([128, 1024], f32, name="pv", tag="pv", bufs=2)
            self.P = {}

        def a0(self, ki):
            return max(ki * 128 - self.q0, 0)

        def spans(self, ki):
            a0 = self.a0(ki)
            return [(a0, 512), (512, 1024)] if a0 < 512 else [(a0, 1024)]

        def emit_scores(self, ki):
            q0, r0 = self.q0, self.r0
            qt, kt = qt_t[self.hp], kt_t[self.hp]
            self.sc = scp.tile([128, 1024], f32, name="sc", tag="sc", bufs=2)
            for (a, b) in self.spans(ki):
                nc.tensor.matmul(
                    self.sc[:, a:b],
                    kt[r0:r0 + 64, ki * 128:(ki + 1) * 128],
                    qt[r0:r0 + 64, q0 + a:q0 + b],
                    start=True, stop=True,
                )

        def emit_exp(self, ki):
            a0 = self.a0(ki)
            P = p_pool.tile([128, 1024], bf16, name="P", tag="P", bufs=6)
            self.P[ki] = P
            nc.scalar.activation(P[:, a0:1024], self.sc[:, a0:1024], AF.Exp,
                                 scale=float(HD) ** -0.5)
            if ki * 128 >= self.q0:  # causal mask on the diagonal block
                nc.vector.tensor_mul(P[:, a0:a0 + 128],
                                     P[:, a0:a0 + 128], mask_t[:])

        def emit_pv(self, ki):
            P = self.P.pop(ki)
            for (a, b) in self.spans(ki):
                last_ki = min(self.ki_n - 1, (self.q0 + b - 1) // 128)
                nc.tensor.matmul(
                    self.pv[0:65, a:b],
                    v_t[ki][:, self.h * 65:self.h * 65 + 65],
                    P[:, a:b],
                    start=(ki == 0), stop=(ki == last_ki),
                )

        def finish_cols(self, c0, c1):
            """Normalize q columns [q0+c0, q0+c1) once their pv is final."""
            pv = self.pv
            n = c1 - c0
            dcp = rc_pool.tile([1, 512], f32, name="dcp", tag="dcp", bufs=4)
            nc.vector.tensor_copy(dcp[:, 0:n], pv[64:65, c0:c1])
            rcp = rc_pool.tile([1, 512], f32, name="rcp", tag="rcp", bufs=4)
            nc.vector.reciprocal_approx_fast(rcp[:, 0:n], dcp[:, 0:n])
            rbc = rc_pool.tile([64, 512], f32, name="rbc", tag="rbc", bufs=4)
            nc.gpsimd.partition_broadcast(rbc[:, 0:n], rcp[:, 0:n], channels=64)
            nc.vector.tensor_mul(
                ot_t[self.hp][self.r0:self.r0 + 64,
                              self.q0 + c0:self.q0 + c1],
                pv[0:64, c0:c1], rbc[:, 0:n],
            )

    def attn_pair(units, fillers=()):
        """Two heads interleaved at ki granularity. PV runs one ki behind
        scores/exp so the ACT engine never waits on a PV head-of-line stall.
        fillers[ki] thunks are emitted between exp(ki) and pv(ki-1)."""
        ua, ub = units
        n = ua.ki_n
        for ki in range(n):
            # adjacent scores land in different PE row groups (heads at
            # partition 0 and 64) and execute concurrently in the array
            ua.emit_scores(ki)
            ub.emit_scores(ki)
            ua.emit_exp(ki)
            ub.emit_exp(ki)
            if ki < len(fillers):
                for fn in fillers[ki]:
                    fn()
            if ki > 0:
                ua.emit_pv(ki - 1)
                ub.emit_pv(ki - 1)
        ua.emit_pv(n - 1)
        ub.emit_pv(n - 1)
        return units

    def proj_group(s):
        pj = scp.tile([128, 1024], f32, name="sc", tag="sc", bufs=2)
        for nh in range(2):
            for k2 in range(2):
                nc.tensor.matmul(
                    pj[:, nh * 512:(nh + 1) * 512],
                    ot_t[k2][:, s * 128:(s + 1) * 128],
                    wp_t[k2][:, nh * 512:(nh + 1) * 512],
                    start=(k2 == 0), stop=(k2 == 1),
                )
        oo = oo_pool.tile([128, D], bf16, name="oo", tag="oo", bufs=3)
        nc.vector.tensor_copy(oo[:], pj[:])
        nc.sync.dma_start(out[s * 128:(s + 1) * 128, :], oo[:])

    def fin(units, c0, c1):
        def f():
            for u in units:
                u.finish_cols(c0, c1)
        return f

    # ---- program order = scheduler priority.
    # Prelude (DMA-paced, HAM-warmed by dummies): f0/c2=0 Q^T,K^T + V[0,1].
    WARM = 0
    if WARM:
        warm(WARM)
    qkt_sp(qt_t, wq_t, 0, 0, 0, 0)
    qkt_sp(qt_t, wq_t, 0, 0, 0, 1)
    qkt_sp(kt_t, wk_t, 2, 0, 0, 0)
    qkt_sp(kt_t, wk_t, 2, 0, 0, 1)
    v_group(0)
    v_group(1)

    q_sp = partial(qkt_sp, qt_t, wq_t, 0)
    k_sp = partial(qkt_sp, kt_t, wk_t, 2)

    # pair(0,1,0): V[2..7] one ki ahead of use; f1/c2=0 for pair(2,3,0).
    # Its pv cols 0:512 are final once pv(3) lands (loop index 4).
    p01_0 = (AttnUnit(0, 0), AttnUnit(1, 0))
    attn_pair(p01_0, fillers=[
        [partial(v_group, 2), partial(v_group, 3)],
        [partial(v_group, 4), partial(v_group, 5)],
        [partial(v_group, 6), partial(v_group, 7)],
        [partial(q_sp, 1, 0, 0)],
        [partial(q_sp, 1, 0, 1)],
        [fin(p01_0, 0, 512), partial(k_sp, 1, 0, 0)],
        [partial(k_sp, 1, 0, 1)],
        [],
    ])
    fin(p01_0, 512, 1024)()

    # pair(2,3,0): V[8..15] for the half-1 pairs; f0/c2=1 Q^T for pair(0,1,1).
    p23_0 = (AttnUnit(2, 0), AttnUnit(3, 0))
    attn_pair(p23_0, fillers=[
        [partial(v_group, 8), partial(v_group, 9)],
        [partial(v_group, 10), partial(v_group, 11)],
        [partial(v_group, 12), partial(v_group, 13)],
        [partial(v_group, 14), partial(v_group, 15)],
        [partial(q_sp, 0, 1, 0)],
        [fin(p23_0, 0, 512), partial(q_sp, 0, 1, 1)],
        [partial(k_sp, 0, 1, 0)],
        [partial(k_sp, 0, 1, 1)],
    ])
    fin(p23_0, 512, 1024)()

    # pair(0,1,1): proj(0..7) of the finished half 0 + f1/c2=1 for pair(2,3,1);
    # its pv cols 0:512 are final once pv(11) lands (loop index 12).
    p01_1 = (AttnUnit(0, 1), AttnUnit(1, 1))
    attn_pair(p01_1, fillers=[
        [partial(proj_group, 0)],
        [partial(proj_group, 1)],
        [partial(proj_group, 2)],
        [partial(proj_group, 3)],
        [partial(proj_group, 4)],
        [partial(proj_group, 5)],
        [partial(proj_group, 6)],
        [partial(proj_group, 7)],
        [partial(q_sp, 1, 1, 0)],
        [partial(q_sp, 1, 1, 1)],
        [partial(k_sp, 1, 1, 0)],
        [partial(k_sp, 1, 1, 1)],
        [],
        [fin(p01_1, 0, 512)],
        [], [],
    ])
    # must precede pair(2,3,1): its pv slots are released by this finish
    fin(p01_1, 512, 1024)()

    # pair(2,3,1): finish+proj(8..11) weave in once cols 0:512 are final
    # (pv(11) is emitted at loop index 12, so the finish goes at index 13).
    p23_1 = (AttnUnit(2, 1), AttnUnit(3, 1))
    attn_pair(p23_1, fillers=[
        [], [], [], [], [], [], [], [],
        [], [], [], [], [],
        [fin(p23_1, 0, 512), partial(proj_group, 8)],
        [partial(proj_group, 9), partial(proj_group, 10)],
        [partial(proj_group, 11)],
    ])

    # tail: last column halves + proj(12..15)
    fin(p23_1, 512, 1024)()
    for s in range(12, NS):
        proj_group(s)


def _in_maps(x, W_qkv, b_qkv, W_proj):
    bf = ml_dtypes.bfloat16
    maps = []
    # multiplicative causal mask for the transposed diag block: keep k<=q
    mask = np.triu(np.ones((128, 128), np.float32)).astype(bf)
    for core in range(NCORES):
        b, hg = core // 4, core % 4
        cs = slice(hg * FPC, (hg + 1) * FPC)
        bq = b_qkv[cs].astype(np.float32)
        bk = b_qkv[D + hg * FPC: D + (hg + 1) * FPC].astype(np.float32)
        wqkv = np.concatenate([
            W_qkv[:, cs],
            W_qkv[:, D + hg * FPC: D + (hg + 1) * FPC],
            W_qkv[:, 2 * D + hg * FPC: 2 * D + (hg + 1) * FPC],
        ], axis=1)
        maps.append({
            "xT": np.ascontiguousarray(x[b].T).astype(bf),
            "wqkv": np.ascontiguousarray(wqkv).astype(bf),
            "wp": np.ascontiguousarray(W_proj[hg * FPC:(hg + 1) * FPC, :]).astype(bf),
            "bqk": np.ascontiguousarray(
                np.stack([bq[0:128], bq[128:256], bk[0:128], bk[128:256]], axis=1)),
            "maskT": mask,
        })
    return maps


def get_nc():
    if "nc" not in _CACHE:
        _CACHE["nc"] = _build()
    return _CACHE["nc"]


def _postprocess(partials, b_qkv, W_proj, b_proj):
    out = np.zeros((B, S, D), np.float32)
    for core in range(NCORES):
        out[core // 4] += np.asarray(partials[core], dtype=np.float32)
    bv = np.asarray(b_qkv, np.float32)[2 * D:3 * D]
    out += bv @ np.asarray(W_proj, np.float32) + np.asarray(b_proj, np.float32)
    return out


def kernel(x, W_qkv, b_qkv, W_proj, b_proj, _trace=False):
    from concourse.bass_utils import run_bass_kernel_spmd

    x = np.asarray(x, np.float32)
    W_qkv = np.asarray(W_qkv, np.float32)
    b_qkv = np.asarray(b_qkv, np.float32)
    W_proj = np.asarray(W_proj, np.float32)
    b_proj = np.asarray(b_proj, np.float32)

    nc = get_nc()
    maps = _in_maps(x, W_qkv, b_qkv, W_proj)
    res = run_bass_kernel_spmd(nc, maps, list(range(NCORES)), trace=_trace)
    _CACHE["last_result"] = res
    partials = [res.results[c]["out"] for c in range(NCORES)]
    return _postprocess(partials, b_qkv, W_proj, b_proj)


# revision 15
# speedup vs baseline: 1.0861x; 1.0694x over previous
"""Causal multi-head attention (B=2, S=2048, D=1024, H=16, hd=64) on 8 trn2 cores.

Sharding: core c handles batch b = c//4 and head group hg = c%4 (4 heads each).
Each core computes its Q/K/V shard (tensor-parallel columns of W_qkv), causal
attention for its 4 heads with scores held transposed ([s_k, s_q] so the PV
matmul needs no on-chip transposes), and a partial output projection over its
256 rows of W_proj. The host sums the 4 partials per batch and adds the exact
bias terms (softmax rows sum to 1, so attn@(V + 1 bv^T) = attn@V + bv^T; the
b_qkv V-slice and b_proj are applied on the host).

v3 schedule:
- inputs stream in k-chunks ordered by first use (qk-f0 weights + x half 0
  first); the host packs W_qkv columns as [qk_f0 | wv | qk_f1] per core so the
  critical prefix is contiguous and minimal.
- dummy matmuls (prelude) and standalone LDWEIGHTS (filler-thin attention
  iterations) keep the PE HAM clock gate at 8/8 so nothing runs at half clock.
- attention pairs software-pipeline the PV matmuls one ki behind scores/exp:
  the next exp never queues behind a PV waiting on the current exp.
- PE filler work (QKV f1 projections, V groups, half-width output-projection
  groups) is spread across iterations at <=1.7us granularity, weighted toward
  the late pairs.
- finishes (softmax normalize) are split by column range and emitted as soon
  as the psum columns stop accumulating, so the output projection of finished
  q-tiles overlaps the last attention pair and the tail is ~6us.
- proj results are cast to bf16 (host accumulates in f32), halving out-DMA.
"""

import numpy as np
import ml_dtypes
from contextlib import ExitStack
from functools import partial

B, S, D, H = 2, 2048, 1024, 16
HD = 64
NCORES = 8
FPC = 256  # features per core (4 heads x 64)

_CACHE = {}


def _build():
    import concourse.bacc as bacc
    import concourse.tile as tile
    import concourse.mybir as mybir

    f32 = mybir.dt.float32
    bf16 = mybir.dt.bfloat16

    nc = bacc.Bacc("TRN2", target_bir_lowering=False, debug=False, num_devices=NCORES)

    xT = nc.dram_tensor("xT", [D, S], bf16, kind="ExternalInput").ap()
    wqkv = nc.dram_tensor("wqkv", [D, 3 * FPC], bf16, kind="ExternalInput").ap()
    wp = nc.dram_tensor("wp", [FPC, D], bf16, kind="ExternalInput").ap()
    bqk = nc.dram_tensor("bqk", [128, 4], f32, kind="ExternalInput").ap()
    maskT = nc.dram_tensor("maskT", [128, 128], bf16, kind="ExternalInput").ap()
    out = nc.dram_tensor("out", [S, D], bf16, kind="ExternalOutput").ap()

    with tile.TileContext(nc) as tc:
        with ExitStack() as ctx:
            _body(ctx, tc, mybir, out, xT, wqkv, wp, bqk, maskT)

    nc.compile()
    return nc


def _body(ctx, tc, mybir, out, xT, wqkv, wp, bqk, maskT):
    nc = tc.nc
    f32 = mybir.dt.float32
    bf16 = mybir.dt.bfloat16
    AF = mybir.ActivationFunctionType
    NK = D // 128   # 8 contraction tiles for qkv/proj-input dim
    NS = S // 128   # 16 sequence tiles

    sb = ctx.enter_context(tc.tile_pool(name="sb", bufs=1))

    xt_a = sb.tile([128, NK * S], bf16, name="xta", tag="xta")
    xt_t = [xt_a[:, k * S:(k + 1) * S] for k in range(NK)]
    # per k-block: [wq_f0 wk_f0 (256) | wv (256) | wq_f1 wk_f1 (256)]
    w_a = sb.tile([128, NK * 3 * FPC], bf16, name="wa", tag="wa")

    def wq_f(k, f):
        o = k * 768 + f * 512
        return w_a[:, o:o + 128]

    def wk_f(k, f):
        o = k * 768 + f * 512 + 128
        return w_a[:, o:o + 128]

    wv_t = [w_a[:, k * 768 + 256: k * 768 + 512] for k in range(NK)]
    wp_a = sb.tile([128, 2 * D], bf16, name="wpa", tag="wpa")
    wp_t = [wp_a[:, k * D:(k + 1) * D] for k in range(2)]
    qt_t = [sb.tile([128, S], bf16, name=f"qtt{f}", tag=f"qtt{f}") for f in range(2)]
    kt_t = [sb.tile([128, S], bf16, name=f"ktt{f}", tag=f"ktt{f}") for f in range(2)]
    v_t = [sb.tile([128, 4 * 65], bf16, name=f"vt{s}", tag=f"vt{s}") for s in range(NS)]
    ot_t = [sb.tile([128, S], bf16, name=f"ott{f}", tag=f"ott{f}") for f in range(2)]
    bqk_t = sb.tile([128, 4], f32, name="bqkt", tag="bqkt")
    mask_t = sb.tile([128, 128], bf16, name="maskt", tag="maskt")

    p_pool = ctx.enter_context(tc.tile_pool(name="pp", bufs=6))
    rc_pool = ctx.enter_context(tc.tile_pool(name="rcp", bufs=2))
    oo_pool = ctx.enter_context(tc.tile_pool(name="oop", bufs=4))

    # ---- input DMAs: chunked per k-tile, in the order compute consumes them.
    nc.sync.dma_start(mask_t[:], maskT[:])
    nc.sync.dma_start(bqk_t[:], bqk[:])
    w4 = wqkv.rearrange("(k p) (g c) -> p k g c", p=128, g=3)
    wa4 = w_a.rearrange("p (k g c) -> p k g c", k=NK, g=3)
    x3 = xT.rearrange("(k p) s -> p k s", p=128)
    xa3 = xt_a.rearrange("p (k s) -> p k s", k=NK)
    for k in range(NK):
        nc.sync.dma_start(wa4[:, k:k + 1, 0:1, :], w4[:, k:k + 1, 0:1, :])
        nc.sync.dma_start(xa3[:, k:k + 1, 0:1024], x3[:, k:k + 1, 0:1024])
    for k in range(NK):
        nc.sync.dma_start(wa4[:, k:k + 1, 1:2, :], w4[:, k:k + 1, 1:2, :])
    for k in range(NK):
        nc.sync.dma_start(wa4[:, k:k + 1, 2:3, :], w4[:, k:k + 1, 2:3, :])
    for k in range(NK):
        nc.sync.dma_start(xa3[:, k:k + 1, 1024:2048], x3[:, k:k + 1, 1024:2048])
    nc.sync.dma_start(wp_a.rearrange("p (k f) -> p k f", k=2),
                      wp.rearrange("(k p) f -> p k f", p=128))

    # PSUM: "sc" slots 2 banks x2 (scores/qkv/v/proj rotation), "pv" 2 banks x2.
    scp = ctx.enter_context(tc.tile_pool(name="ps_sc", bufs=2, space="PSUM"))
    pvp = ctx.enter_context(tc.tile_pool(name="ps_pv", bufs=2, space="PSUM"))

    # ---- HAM warmup. warm(): dummy matmuls into a borrowed pv-pool slot
    # (prelude only; retired long before the first attention pair).
    # lw(): standalone LDWEIGHTS — PE-busy, touches no psum — used to pad
    # filler-thin attention iterations so the clock gate never closes.
    def warm(n):
        warm_t = pvp.tile([128, 1024], f32, name="pv", tag="pv", bufs=2)
        for _ in range(n):
            nc.tensor.matmul(warm_t[:, 0:128], mask_t[:], mask_t[:],
                             start=True, stop=True)

    def lw(n):
        def f():
            for _ in range(n):
                nc.tensor.ldweights(mask_t[:])
        return f

    def qkt_sp(dst, wsel, bcol, f, c2, sp, interleave=0):
        """Half (512 q cols) of one [128,1024] Q^T/K^T projection group."""
        wf = wq_f if wsel == "q" else wk_f
        ps = scp.tile([128, 1024], f32, name="sc", tag="sc", bufs=2)
        for k in range(NK):
            if interleave:
                warm(interleave)
            nc.tensor.matmul(
                ps[:, sp * 512:(sp + 1) * 512],
                wf(k, f),
                xt_t[k][:, c2 * 1024 + sp * 512: c2 * 1024 + (sp + 1) * 512],
                start=(k == 0), stop=(k == NK - 1),
            )
        nc.vector.tensor_scalar_add(
            dst[f][:, c2 * 1024 + sp * 512:c2 * 1024 + (sp + 1) * 512],
            ps[:, sp * 512:(sp + 1) * 512],
            bqk_t[:, bcol + f: bcol + f + 1],
        )

    def v_group(s):
        psv = scp.tile([128, FPC], f32, name="sc", tag="sc", bufs=2)
        for k in range(NK):
            nc.tensor.matmul(
                psv[:],
                xt_t[k][:, s * 128:(s + 1) * 128],
                wv_t[k][:],
                start=(k == 0), stop=(k == NK - 1),
            )
        v3 = v_t[s].rearrange("p (h c) -> p h c", h=4)
        nc.vector.tensor_copy(v3[:, :, 0:64],
                              psv.rearrange("p (h c) -> p h c", h=4)[:])
        nc.vector.memset(v3[:, :, 64:65], 1.0)

    class AttnUnit:
        """Causal attention for head h over queries [half*1024, +1024)."""

        def __init__(self, h, half):
            self.h, self.half = h, half
            self.hp, self.hh = h // 2, h % 2
            self.r0 = self.hh * 64
            self.q0 = half * 1024
            self.ki_n = NS // 2 * (half + 1)
            self.pv = pvp.tile([128, 1024], f32, name="pv", tag="pv", bufs=2)
            self.P = {}

        def a0(self, ki):
            return max(ki * 128 - self.q0, 0)

        def spans(self, ki):
            a0 = self.a0(ki)
            return [(a0, 512), (512, 1024)] if a0 < 512 else [(a0, 1024)]

        def emit_scores(self, ki):
            q0, r0 = self.q0, self.r0
            qt, kt = qt_t[self.hp], kt_t[self.hp]
            self.sc = scp.tile([128, 1024], f32, name="sc", tag="sc", bufs=2)
            for (a, b) in self.spans(ki):
                nc.tensor.matmul(
                    self.sc[:, a:b],
                    kt[r0:r0 + 64, ki * 128:(ki + 1) * 128],
                    qt[r0:r0 + 64, q0 + a:q0 + b],
                    start=True, stop=True,
                )

        def emit_exp(self, ki):
            a0 = self.a0(ki)
            P = p_pool.tile([128, 1024], bf16, name="P", tag="P", bufs=6)
            self.P[ki] = P
            nc.scalar.activation(P[:, a0:1024], self.sc[:, a0:1024], AF.Exp,
                                 scale=float(HD) ** -0.5)
            if ki * 128 >= self.q0:  # causal mask on the diagonal block
                nc.vector.tensor_mul(P[:, a0:a0 + 128],
                                     P[:, a0:a0 + 128], mask_t[:])

        def emit_pv(self, ki):
            P = self.P.pop(ki)
            for (a, b) in self.spans(ki):
                last_ki = min(self.ki_n - 1, (self.q0 + b - 1) // 128)
                nc.tensor.matmul(
                    self.pv[0:65, a:b],
                    v_t[ki][:, self.h * 65:self.h * 65 + 65],
                    P[:, a:b],
                    start=(ki == 0), stop=(ki == last_ki),
                )

        def finish_cols(self, c0, c1):
            """Normalize q columns [q0+c0, q0+c1) once their pv is final."""
            pv = self.pv
            n = c1 - c0
            dcp = rc_pool.tile([1, 512], f32, name="dcp", tag="dcp", bufs=4)
            nc.vector.tensor_copy(dcp[:, 0:n], pv[64:65, c0:c1])
            rcp = rc_pool.tile([1, 512], f32, name="rcp", tag="rcp", bufs=4)
            nc.vector.reciprocal_approx_fast(rcp[:, 0:n], dcp[:, 0:n])
            rbc = rc_pool.tile([64, 512], f32, name="rbc", tag="rbc", bufs=4)
            nc.gpsimd.partition_broadcast(rbc[:, 0:n], rcp[:, 0:n], channels=64)
            nc.vector.tensor_mul(
                ot_t[self.hp][self.r0:self.r0 + 64,
                              self.q0 + c0:self.q0 + c1],
                pv[0:64, c0:c1], rbc[:, 0:n],
            )

    def attn_pair(units, fillers=()):
        """Two heads interleaved at ki granularity. PV runs one ki behind
        scores/exp so the ACT engine never waits on a PV head-of-line stall.
        fillers[ki] thunks are emitted between exp(ki) and pv(ki-1)."""
        ua, ub = units
        n = ua.ki_n
        for ki in range(n):
            # adjacent scores land in different PE row groups (heads at
            # partition 0 and 64) and execute concurrently in the array
            ua.emit_scores(ki)
            ub.emit_scores(ki)
            ua.emit_exp(ki)
            ub.emit_exp(ki)
            if ki < len(fillers):
                for fn in fillers[ki]:
                    fn()
            if ki > 0:
                ua.emit_pv(ki - 1)
                ub.emit_pv(ki - 1)
        ua.emit_pv(n - 1)
        ub.emit_pv(n - 1)
        return units

    def proj_h(s, nh):
        """Half-width (512 features) output-projection of q-tile s."""
        pj = scp.tile([128, 1024], f32, name="sc", tag="sc", bufs=2)
        for k2 in range(2):
            nc.tensor.matmul(
                pj[:, nh * 512:(nh + 1) * 512],
                ot_t[k2][:, s * 128:(s + 1) * 128],
                wp_t[k2][:, nh * 512:(nh + 1) * 512],
                start=(k2 == 0), stop=(k2 == 1),
            )
        oo = oo_pool.tile([128, 512], bf16, name="oo", tag="oo", bufs=4)
        nc.vector.tensor_copy(oo[:], pj[:, nh * 512:(nh + 1) * 512])
        nc.sync.dma_start(out[s * 128:(s + 1) * 128, nh * 512:(nh + 1) * 512],
                          oo[:])

    def ph(s):
        return [partial(proj_h, s, 0), partial(proj_h, s, 1)]

    def fin(units, c0, c1):
        def f():
            for u in units:
                u.finish_cols(c0, c1)
        return f

    q_sp = partial(qkt_sp, qt_t, "q", 0)
    k_sp = partial(qkt_sp, kt_t, "k", 2)

    # ---- program order = scheduler priority.
    # Prelude (DMA-paced; dummies keep the PE dense): f0/c2=0 Q^T,K^T + V[0,1].
    warm(40)
    qkt_sp(qt_t, "q", 0, 0, 0, 0, interleave=3)
    qkt_sp(qt_t, "q", 0, 0, 0, 1, interleave=3)
    qkt_sp(kt_t, "k", 2, 0, 0, 0, interleave=2)
    qkt_sp(kt_t, "k", 2, 0, 0, 1)
    v_group(0)
    v_group(1)

    # pair(0,1,0): V[2..7] one ki ahead of use; f1/c2=0 for pair(2,3,0).
    # pv cols 0:512 are final once pv(3) lands (emitted at loop index 4).
    p01_0 = (AttnUnit(0, 0), AttnUnit(1, 0))
    attn_pair(p01_0, fillers=[
        [partial(v_group, 2), partial(v_group, 3)],
        [partial(v_group, 4), partial(v_group, 5)],
        [partial(v_group, 6), partial(v_group, 7)],
        [partial(q_sp, 1, 0, 0)],
        [partial(q_sp, 1, 0, 1)],
        [fin(p01_0, 0, 512), partial(k_sp, 1, 0, 0)],
        [lw(4)],
        [lw(4)],
    ])
    fin(p01_0, 512, 1024)()

    # pair(2,3,0): V[8..13] for the half-1 pairs; f0/c2=1 Q^T for pair(0,1,1);
    # kt f1/c2=0 sp1 is only needed from this pair's own ki=4.
    p23_0 = (AttnUnit(2, 0), AttnUnit(3, 0))
    attn_pair(p23_0, fillers=[
        [partial(k_sp, 1, 0, 1)],
        [partial(v_group, 8), partial(v_group, 9)],
        [partial(v_group, 10), partial(v_group, 11)],
        [partial(v_group, 12), partial(v_group, 13)],
        [partial(q_sp, 0, 1, 0)],
        [fin(p23_0, 0, 512), partial(q_sp, 0, 1, 1)],
        [lw(4)],
        [lw(4)],
    ])
    fin(p23_0, 512, 1024)()

    # pair(0,1,1): proj(0..3) of the finished half 0, kt f0/c2=1 for its own
    # ki>=8, f1/c2=1 Q^T for pair(2,3,1), V[14,15] (used from ki 14).
    p01_1 = (AttnUnit(0, 1), AttnUnit(1, 1))
    attn_pair(p01_1, fillers=[
        [partial(v_group, 14), partial(v_group, 15)],
        [partial(k_sp, 0, 1, 0)],
        [partial(k_sp, 0, 1, 1)],
        ph(0),
        ph(1),
        ph(2),
        ph(3),
        [partial(q_sp, 1, 1, 0)],
        [partial(q_sp, 1, 1, 1)],
        [lw(8)],
        [lw(8)],
        [lw(8)],
        [lw(8)],
        [fin(p01_1, 0, 512), lw(6)],
        [lw(8)],
        [lw(8)],
    ])
    # must precede pair(2,3,1): its pv slots are released by this finish
    fin(p01_1, 512, 1024)()

    # pair(2,3,1): proj(4..7) + its own kt f1/c2=1; once cols 0:512 are final
    # (pv(11) emitted at loop index 12) finish+proj(8..11) weave in; cols
    # 512:768 are final after pv(13) (index 14) -> finish at index 15.
    p23_1 = (AttnUnit(2, 1), AttnUnit(3, 1))
    attn_pair(p23_1, fillers=[
        [partial(k_sp, 1, 1, 0)],
        ph(4),
        [partial(k_sp, 1, 1, 1)],
        ph(5),
        ph(6),
        ph(7),
        [lw(8)],
        [lw(8)],
        [lw(8)],
        [lw(8)],
        [lw(8)],
        [lw(8)],
        [lw(8)],
        [fin(p23_1, 0, 512)] + ph(8),
        ph(9) + ph(10),
        [fin(p23_1, 512, 768)] + ph(11),
    ])

    # tail: proj 12,13 (cols 512:768 finished at loop index 15), last finish,
    # proj 14,15.
    for t in ph(12) + ph(13):
        t()
    fin(p23_1, 768, 1024)()
    for t in ph(14) + ph(15):
        t()


def _in_maps(x, W_qkv, b_qkv, W_proj):
    bf = ml_dtypes.bfloat16
    maps = []
    # multiplicative causal mask for the transposed diag block: keep k<=q
    mask = np.triu(np.ones((128, 128), np.float32)).astype(bf)
    for core in range(NCORES):
        b, hg = core // 4, core % 4
        q = W_qkv[:, hg * FPC:(hg + 1) * FPC]
        k = W_qkv[:, D + hg * FPC: D + (hg + 1) * FPC]
        v = W_qkv[:, 2 * D + hg * FPC: 2 * D + (hg + 1) * FPC]
        bq = b_qkv[hg * FPC:(hg + 1) * FPC].astype(np.float32)
        bk = b_qkv[D + hg * FPC: D + (hg + 1) * FPC].astype(np.float32)
        # column order per k-block row group: [q_f0 k_f0 | v | q_f1 k_f1]
        wpack = np.concatenate(
            [q[:, 0:128], k[:, 0:128], v, q[:, 128:256], k[:, 128:256]], axis=1)
        maps.append({
            "xT": np.ascontiguousarray(x[b].T).astype(bf),
            "wqkv": np.ascontiguousarray(wpack).astype(bf),
            "wp": np.ascontiguousarray(W_proj[hg * FPC:(hg + 1) * FPC, :]).astype(bf),
            "bqk": np.ascontiguousarray(
                np.stack([bq[0:128], bq[128:256], bk[0:128], bk[128:256]], axis=1)),
            "maskT": mask,
        })
    return maps


def get_nc():
    if "nc" not in _CACHE:
        _CACHE["nc"] = _build()
    return _CACHE["nc"]


def _postprocess(partials, b_qkv, W_proj, b_proj):
    out = np.zeros((B, S, D), np.float32)
    for core in range(NCORES):
        out[core // 4] += np.asarray(partials[core], dtype=np.float32)
    bv = np.asarray(b_qkv, np.float32)[2 * D:3 * D]
    out += bv @ np.asarray(W_proj, np.float32) + np.asarray(b_proj, np.float32)
    return out


def kernel(x, W_qkv, b_qkv, W_proj, b_proj, _trace=False):
    from concourse.bass_utils import run_bass_kernel_spmd

    x = np.asarray(x, np.float32)
    W_qkv = np.asarray(W_qkv, np.float32)
    b_qkv = np.asarray(b_qkv, np.float32)
    W_proj = np.asarray(W_proj, np.float32)
    b_proj = np.asarray(b_proj, np.float32)

    nc = get_nc()
    maps = _in_maps(x, W_qkv, b_qkv, W_proj)
    res = run_bass_kernel_spmd(nc, maps, list(range(NCORES)), trace=_trace)
    _CACHE["last_result"] = res
    partials = [res.results[c]["out"] for c in range(NCORES)]
    return _postprocess(partials, b_qkv, W_proj, b_proj)
